# revision 19
# baseline (speedup 1.0000x reference)
"""Attention-pooling kernel for Trainium2 (raw Bass), SPMD over 8 NeuronCores.

Computation (per sample b):
    score[t] = tanh(sum_d X[b,t,d] * w[d] + bias[t])
    out[b,d] = sum_t softmax_t(score)[t] * X[b,t,d]

Sharding: data-parallel over batch (32 samples -> 4 per core); w/bias replicated.

Per-core dataflow (4 samples, T=2048 as 16 chunks of 128 partitions, X kept in
natural [t-part, d-free] layout, loaded once from HBM = the memory roofline):
  - X is loaded by SWDGE in 4 x 2MiB pieces per sample, cast f32 -> float32r
    in the DMA so the PE can later consume it single-pass (4x the fp32 matmul
    rate; ~1e-4 relative rounding, measured 9e-5 end-to-end vs fp32 reference).
  - score: DVE scalar_tensor_tensor fuses the X*w multiply with the free-dim
    sum via accum_out (the product tensor goes to a broadcast scrap AP, never
    materialized). One DVE pass over X total; w is pre-broadcast to all 128
    partitions once.
  - softmax without max-subtraction (tanh output is in [-1,1], exp is safe):
    per 4-chunk group, ACT tanh then exp with fused accum_out row-sums; PE
    accumulates the cross-partition total in PSUM via per-group matmuls
    against a ones column; DVE reciprocal. Working per-group lets pooling
    start while later groups are still being scored.
  - pooling: PE matmuls escore[:,c].T @ X_chunk accumulated in PSUM [1,1024];
    the 1/sum(exp) normalization is folded into the PSUM->SBUF copy as the
    ACT per-partition scale.
Everything is double-buffered by sample parity; engines sync with explicit
semaphores (written raw because this container's neuronxcc rejects the
Tile-layer's generated sync). Each DMA slot gets its own semaphore: two DMAs
incrementing one semaphore can interleave their 16 per-SDMA-engine updates, so
a cumulative wait would not guarantee the first transfer landed.
Steady-state is DMA-bound at the measured ~460 GB/s/core (~73 us per full
pass; 32 MiB of reads per core).
"""

import numpy as np

import concourse.bass as bass
import concourse.mybir as mybir
from concourse.bass_utils import run_bass_kernel_spmd

B, T, D = 32, 2048, 1024
N_CORES = 8
BPC = B // N_CORES  # samples per core
P = 128
NCHUNK = T // P  # 16
NGROUP = 4  # input DMAs per sample (2 MiB each)
CPG = NCHUNK // NGROUP  # chunks per DMA group

f32 = mybir.dt.float32
f32r = mybir.dt.float32r
# float32r runs the pooling matmuls single-pass (4x the fp32 rate) at slightly
# reduced multiply precision; flip to False to fall back to exact fp32.
POOL_F32R = True
Tanh = mybir.ActivationFunctionType.Tanh
Exp = mybir.ActivationFunctionType.Exp
Copy = mybir.ActivationFunctionType.Copy
Alu = mybir.AluOpType


def _build_nc(reps: int = 1) -> bass.Bass:
    nc = bass.Bass("TRN2", target_bir_lowering=False, debug=False)
    x = nc.dram_tensor("x", [BPC, T, D], f32, kind="ExternalInput").ap()
    w = nc.dram_tensor("w", [D, 1], f32, kind="ExternalInput")
    bias = nc.dram_tensor("bias", [T, 1], f32, kind="ExternalInput")
    out = nc.dram_tensor("out", [BPC, D], f32, kind="ExternalOutput").ap()

    NS = BPC * reps  # virtual sample count (reps>1 re-runs the pipeline for timing)

    from contextlib import ExitStack

    with ExitStack() as es:
        ec = es.enter_context
        xt_dt = f32r if POOL_F32R else f32
        xt0 = ec(nc.sbuf_tensor("xt0", [P, NCHUNK, D], xt_dt))
        xt1 = ec(nc.sbuf_tensor("xt1", [P, NCHUNK, D], xt_dt))
        wt = ec(nc.sbuf_tensor("wt", [P, D], f32))
        bias_t = ec(nc.sbuf_tensor("bias_t", [P, NCHUNK], f32))
        ones_col = ec(nc.sbuf_tensor("ones_col", [P, 1], f32))
        scrap = ec(nc.sbuf_tensor("scrap", [P, 2 * NCHUNK], f32))
        score0 = ec(nc.sbuf_tensor("score0", [P, NCHUNK], f32))
        score1 = ec(nc.sbuf_tensor("score1", [P, NCHUNK], f32))
        esc0 = ec(nc.sbuf_tensor("esc0", [P, NCHUNK], xt_dt))
        esc1 = ec(nc.sbuf_tensor("esc1", [P, NCHUNK], xt_dt))
        sumexp0 = ec(nc.sbuf_tensor("sumexp0", [P, NGROUP], f32))
        sumexp1 = ec(nc.sbuf_tensor("sumexp1", [P, NGROUP], f32))
        recip0 = ec(nc.sbuf_tensor("recip0", [1, 1], f32))
        recip1 = ec(nc.sbuf_tensor("recip1", [1, 1], f32))
        orow0 = ec(nc.sbuf_tensor("orow0", [1, D], f32))
        orow1 = ec(nc.sbuf_tensor("orow1", [1, D], f32))
        pa0 = ec(nc.psum_tensor("pool_a0", [1, 512], f32))
        pa1 = ec(nc.psum_tensor("pool_a1", [1, 512], f32))
        pb0 = ec(nc.psum_tensor("pool_b0", [1, 512], f32))
        pb1 = ec(nc.psum_tensor("pool_b1", [1, 512], f32))
        tot0 = ec(nc.psum_tensor("tot0", [1, 1], f32))
        tot1 = ec(nc.psum_tensor("tot1", [1, 1], f32))
        cset = ec(nc.semaphore("cset"))
        dma_in_s = [
            [ec(nc.semaphore(f"dma_in{p}{g}")) for g in range(NGROUP)]
            for p in range(2)
        ]
        dve_sem = ec(nc.semaphore("dve_sem"))
        act_sem = ec(nc.semaphore("act_sem"))
        pe_tot = ec(nc.semaphore("pe_tot"))
        recip_sem = ec(nc.semaphore("recip_sem"))
        pe_pool = ec(nc.semaphore("pe_pool"))
        act_out = ec(nc.semaphore("act_out"))
        ones_sem = ec(nc.semaphore("ones_sem"))
        stt_sem = ec(nc.semaphore("stt_sem"))
        act_i_sem = ec(nc.semaphore("act_i_sem"))
        dma_out0 = ec(nc.semaphore("dma_out0"))
        dma_out1 = ec(nc.semaphore("dma_out1"))
        block = ec(nc.Block())
        xt = [xt0, xt1]
        score = [score0, score1]
        esc = [esc0, esc1]
        sumexp = [sumexp0, sumexp1]
        recip = [recip0, recip1]
        orow = [orow0, orow1]
        pa = [pa0, pa1]
        pb = [pb0, pb1]
        tot = [tot0, tot1]
        dma_out_s = [dma_out0, dma_out1]

        # dve_sem counts one per group (incremented by the per-group bias add)

        @block.gpsimd
        def _(gpsimd):
            gpsimd.dma_start(
                wt[:], bass.AP(tensor=w, offset=0, ap=[[0, P], [1, D]])
            ).then_inc(cset, 16)
            with nc.allow_non_contiguous_dma(reason="one-time 8KB bias load"):
                gpsimd.dma_start(
                    bias_t[:], bass.AP(tensor=bias, offset=0, ap=[[1, P], [P, NCHUNK]])
                ).then_inc(cset, 16)
            gpsimd.memset(ones_col[:], 1.0).then_inc(ones_sem, 1)
            for v in range(NS):
                s, p_ = v % BPC, v % 2
                xs = x[s].rearrange("(c p) d -> p c d", p=P)
                if v >= 2:
                    gpsimd.wait_ge(pe_pool, v - 1)  # xt[p_] free (pooling v-2 done)
                for g in range(NGROUP):
                    gpsimd.dma_start(
                        out=xt[p_][:, g * CPG : (g + 1) * CPG, :],
                        in_=xs[:, g * CPG : (g + 1) * CPG, :],
                    ).then_inc(dma_in_s[p_][g], 16)

        @block.sync
        def _(sync):
            for v in range(NS):
                s, p_ = v % BPC, v % 2
                sync.wait_ge(act_out, v + 1)
                sync.dma_start(out=out[s : s + 1, :], in_=orow[p_][:]).then_inc(
                    dma_out_s[p_], 16
                )
            # drain: make sure every output store has landed before the
            # program retires (the runtime may otherwise read early)
            sync.wait_ge(dma_out_s[0], 16 * ((NS + 1) // 2))
            if NS > 1:
                sync.wait_ge(dma_out_s[1], 16 * (NS // 2))

        @block.vector
        def _(vector):
            vector.wait_ge(cset, 32)  # wt + bias_t loaded
            for v in range(NS):
                s, p_ = v % BPC, v % 2
                for g in range(NGROUP):
                    vector.wait_ge(dma_in_s[p_][g], 16 * (v // 2 + 1))
                    if g == 0 and v >= 2:
                        # score[p_] free (all exp groups of v-2 done)
                        vector.wait_ge(act_sem, NGROUP * (v - 1))
                    for c in range(g * CPG, (g + 1) * CPG):
                        sc = p_ * NCHUNK + c
                        nc.vector.scalar_tensor_tensor(
                            out=scrap[:, sc : sc + 1].broadcast_to((P, D)),
                            in0=xt[p_][:, c, :].bitcast(f32),
                            scalar=0.0,
                            in1=wt[:],
                            op0=Alu.bypass,
                            op1=Alu.mult,
                            accum_out=score[p_][:, c : c + 1],
                        ).then_inc(stt_sem, 1)
                    gs = slice(g * CPG, (g + 1) * CPG)
                    vector.wait_ge(stt_sem, NCHUNK * v + (g + 1) * CPG)
                    nc.vector.tensor_tensor(
                        out=score[p_][:, gs],
                        in0=score[p_][:, gs],
                        in1=bias_t[:, gs],
                        op=Alu.add,
                    ).then_inc(dve_sem, 1)
                if v >= 1:
                    pv, pp = v - 1, (v - 1) % 2
                    vector.wait_ge(pe_tot, pv + 1)
                    if pv >= 2:
                        vector.wait_ge(act_out, pv - 1)  # recip[pp] free (copy pv-2)
                    nc.vector.reciprocal(out=recip[pp][:], in_=tot[pp][:]).then_inc(
                        recip_sem, 1
                    )
            pv, pp = NS - 1, (NS - 1) % 2
            vector.wait_ge(pe_tot, pv + 1)
            if pv >= 2:
                vector.wait_ge(act_out, pv - 1)
            nc.vector.reciprocal(out=recip[pp][:], in_=tot[pp][:]).then_inc(
                recip_sem, 1
            )

        def _emit_copies(scalar, v):
            s, p_ = v % BPC, v % 2
            scalar.wait_ge(pe_pool, v + 1)
            scalar.wait_ge(recip_sem, v + 1)
            if v >= 2:
                scalar.wait_ge(dma_out_s[p_], 16 * (v // 2))  # orow[p_] free (out-DMA v-2)
            nc.scalar.activation(
                out=orow[p_][:, 0:512], in_=pa[p_][:], func=Copy, scale=recip[p_][:]
            )
            nc.scalar.activation(
                out=orow[p_][:, 512:1024], in_=pb[p_][:], func=Copy, scale=recip[p_][:]
            ).then_inc(act_out, 1)

        @block.scalar
        def _(scalar):
            for v in range(NS):
                s, p_ = v % BPC, v % 2
                for g in range(NGROUP):
                    gs = slice(g * CPG, (g + 1) * CPG)
                    scalar.wait_ge(dve_sem, NGROUP * v + g + 1)
                    nc.scalar.activation(
                        out=score[p_][:, gs], in_=score[p_][:, gs], func=Tanh
                    ).then_inc(act_i_sem, 1)
                    scalar.wait_ge(act_i_sem, NGROUP * v + g + 1)
                    if g == 0 and v >= 2:
                        scalar.wait_ge(pe_pool, v - 1)  # esc[p_] free (pooling v-2)
                    nc.scalar.activation(
                        out=esc[p_][:, gs],
                        in_=score[p_][:, gs],
                        func=Exp,
                        accum_out=sumexp[p_][:, g : g + 1],
                    ).then_inc(act_sem, 1)
                if v >= 1:
                    _emit_copies(scalar, v - 1)
            _emit_copies(scalar, NS - 1)

        @block.tensor
        def _(tensor):
            tensor.wait_ge(ones_sem, 1)  # ones ready
            for v in range(NS):
                s, p_ = v % BPC, v % 2
                for g in range(NGROUP):
                    tensor.wait_ge(act_sem, NGROUP * v + g + 1)
                    if g == 0 and v >= 2:
                        tensor.wait_ge(recip_sem, v - 1)  # tot[p_] free (recip v-2)
                        tensor.wait_ge(act_out, v - 1)  # pa/pb[p_] free (copies v-2)
                    mm_t = nc.tensor.matmul(
                        tot[p_][:],
                        sumexp[p_][:, g : g + 1],
                        ones_col[:],
                        start=(g == 0),
                        stop=(g == NGROUP - 1),
                    )
                    if g == NGROUP - 1:
                        mm_t.then_inc(pe_tot, 1)
                    for c in range(g * CPG, (g + 1) * CPG):
                        st, sp = c == 0, c == NCHUNK - 1
                        nc.tensor.matmul(
                            pa[p_][:], esc[p_][:, c : c + 1], xt[p_][:, c, 0:512],
                            start=st, stop=sp,
                        )
                        mm = nc.tensor.matmul(
                            pb[p_][:], esc[p_][:, c : c + 1], xt[p_][:, c, 512:1024],
                            start=st, stop=sp,
                        )
                mm.then_inc(pe_pool, 1)

    return nc


_NC_CACHE: dict = {}


def _build(reps: int = 1) -> bass.Bass:
    if reps not in _NC_CACHE:
        _NC_CACHE[reps] = _build_nc(reps)
    return _NC_CACHE[reps]


def _in_maps(x, w, b):
    return [
        {"x": x[c * BPC : (c + 1) * BPC], "w": w, "bias": b} for c in range(N_CORES)
    ]


def kernel(**inputs):
    x = np.ascontiguousarray(np.asarray(inputs["inputs"], dtype=np.float32))
    w = np.ascontiguousarray(np.asarray(inputs["att_weight"], dtype=np.float32))
    b = np.ascontiguousarray(np.asarray(inputs["att_bias"], dtype=np.float32))
    nc = _build()
    res = run_bass_kernel_spmd(nc, _in_maps(x, w, b), list(range(N_CORES)))
    return np.concatenate([r["out"] for r in res.results], axis=0)


# revision 28
# speedup vs baseline: 1.0527x; 1.0527x over previous
"""Attention-pooling kernel for Trainium2 (raw Bass), SPMD over 8 NeuronCores.

Computation (per sample b):
    score[t] = tanh(sum_d X[b,t,d] * w[d] + bias[t])
    out[b,d] = sum_t softmax_t(score)[t] * X[b,t,d]

Sharding: data-parallel over batch (32 samples -> 4 per core); w/bias replicated.

Per-core dataflow (4 samples, T=2048 as 16 chunks of 128 partitions, X kept in
natural [t-part, d-free] layout, loaded once from HBM = the memory roofline):
  - X is loaded by SWDGE in a few large pieces per sample, cast f32 -> float32r
    in the DMA so the PE can later consume it single-pass (4x the fp32 matmul
    rate; ~1e-4 relative rounding, measured 9e-5 end-to-end vs fp32 reference).
  - score: DVE scalar_tensor_tensor fuses the X*w multiply with the free-dim
    sum via accum_out (the product tensor goes to a broadcast scrap AP, never
    materialized). One DVE pass over X total; w is pre-broadcast to all 128
    partitions once.
  - softmax without max-subtraction (tanh output is in [-1,1], exp is safe):
    per 4-chunk group, ACT tanh then exp with fused accum_out row-sums; PE
    accumulates the cross-partition total in PSUM via per-group matmuls
    against a ones column; DVE reciprocal. Working per-group lets pooling
    start while later groups are still being scored.
  - pooling: PE matmuls escore[:,c].T @ X_chunk accumulated in PSUM [1,1024];
    the 1/sum(exp) normalization is folded into the PSUM->SBUF copy as the
    ACT per-partition scale. The 4 output rows are staged side-by-side in one
    SBUF row and written with a single DMA per pass: small dma_starts cost
    ~5-7us of pipeline time each on this hardware, so batching them out was
    worth ~21us/pass (measured).
Everything is double-buffered by sample parity; engines sync with explicit
semaphores (written raw because this container's neuronxcc rejects the
Tile-layer's generated sync). Each DMA slot gets its own semaphore: two DMAs
incrementing one semaphore can interleave their 16 per-SDMA-engine updates, so
a cumulative wait would not guarantee the first transfer landed.
Steady-state is DMA-bound; measured ~65-68 us per full pass (32 MiB of reads
per core). NGROUP=2 is the sweet spot: fewer/bigger input DMAs are faster
(per-dma_start overhead), but a single 8 MiB DMA per sample loses because
compute can only start after the whole sample lands, which starves the
buffer-free gating.
"""

import numpy as np

import concourse.bass as bass
import concourse.mybir as mybir
from concourse.bass_utils import run_bass_kernel_spmd

B, T, D = 32, 2048, 1024
N_CORES = 8
BPC = B // N_CORES  # samples per core
P = 128
NCHUNK = T // P  # 16
NGROUP = 2  # input DMAs per sample (4 MiB each; fewer/bigger DMAs measure faster on HW)
CPG = NCHUNK // NGROUP  # chunks per DMA group

f32 = mybir.dt.float32
f32r = mybir.dt.float32r
# float32r runs the pooling matmuls single-pass (4x the fp32 rate) at slightly
# reduced multiply precision; flip to False to fall back to exact fp32.
POOL_F32R = True
Tanh = mybir.ActivationFunctionType.Tanh
Exp = mybir.ActivationFunctionType.Exp
Copy = mybir.ActivationFunctionType.Copy
Alu = mybir.AluOpType


def _build_nc(reps: int = 1, ngroup: int = NGROUP) -> bass.Bass:
    # locals shadow the module defaults so the body below follows `ngroup`
    NGROUP = ngroup  # noqa: F841 (shadowing on purpose)
    CPG = NCHUNK // ngroup
    nc = bass.Bass("TRN2", target_bir_lowering=False, debug=False)
    x = nc.dram_tensor("x", [BPC, T, D], f32, kind="ExternalInput").ap()
    w = nc.dram_tensor("w", [D, 1], f32, kind="ExternalInput")
    bias = nc.dram_tensor("bias", [T, 1], f32, kind="ExternalInput")
    out = nc.dram_tensor("out", [BPC, D], f32, kind="ExternalOutput").ap()

    NS = BPC * reps  # virtual sample count (reps>1 re-runs the pipeline for timing)

    from contextlib import ExitStack

    with ExitStack() as es:
        ec = es.enter_context
        xt_dt = f32r if POOL_F32R else f32
        xt0 = ec(nc.sbuf_tensor("xt0", [P, NCHUNK, D], xt_dt))
        xt1 = ec(nc.sbuf_tensor("xt1", [P, NCHUNK, D], xt_dt))
        wt = ec(nc.sbuf_tensor("wt", [P, D], f32))
        bias_t = ec(nc.sbuf_tensor("bias_t", [P, NCHUNK], f32))
        ones_col = ec(nc.sbuf_tensor("ones_col", [P, 1], f32))
        scrap = ec(nc.sbuf_tensor("scrap", [P, 2 * NCHUNK], f32))
        score0 = ec(nc.sbuf_tensor("score0", [P, NCHUNK], f32))
        score1 = ec(nc.sbuf_tensor("score1", [P, NCHUNK], f32))
        esc0 = ec(nc.sbuf_tensor("esc0", [P, NCHUNK], xt_dt))
        esc1 = ec(nc.sbuf_tensor("esc1", [P, NCHUNK], xt_dt))
        sumexp0 = ec(nc.sbuf_tensor("sumexp0", [P, NGROUP], f32))
        sumexp1 = ec(nc.sbuf_tensor("sumexp1", [P, NGROUP], f32))
        recip0 = ec(nc.sbuf_tensor("recip0", [1, 1], f32))
        recip1 = ec(nc.sbuf_tensor("recip1", [1, 1], f32))
        orow0 = ec(nc.sbuf_tensor("orow0", [1, BPC * D], f32))
        orow1 = ec(nc.sbuf_tensor("orow1", [1, BPC * D], f32))
        pa0 = ec(nc.psum_tensor("pool_a0", [1, 512], f32))
        pa1 = ec(nc.psum_tensor("pool_a1", [1, 512], f32))
        pb0 = ec(nc.psum_tensor("pool_b0", [1, 512], f32))
        pb1 = ec(nc.psum_tensor("pool_b1", [1, 512], f32))
        tot0 = ec(nc.psum_tensor("tot0", [1, 1], f32))
        tot1 = ec(nc.psum_tensor("tot1", [1, 1], f32))
        cset = ec(nc.semaphore("cset"))
        dma_in_s = [
            [ec(nc.semaphore(f"dma_in{p}{g}")) for g in range(NGROUP)]
            for p in range(2)
        ]
        dve_sem = ec(nc.semaphore("dve_sem"))
        act_sem = ec(nc.semaphore("act_sem"))
        pe_tot = ec(nc.semaphore("pe_tot"))
        recip_sem = ec(nc.semaphore("recip_sem"))
        pe_pool = ec(nc.semaphore("pe_pool"))
        act_out = ec(nc.semaphore("act_out"))
        ones_sem = ec(nc.semaphore("ones_sem"))
        stt_sem = ec(nc.semaphore("stt_sem"))
        act_i_sem = ec(nc.semaphore("act_i_sem"))
        dma_out0 = ec(nc.semaphore("dma_out0"))
        dma_out1 = ec(nc.semaphore("dma_out1"))
        block = ec(nc.Block())
        xt = [xt0, xt1]
        score = [score0, score1]
        esc = [esc0, esc1]
        sumexp = [sumexp0, sumexp1]
        recip = [recip0, recip1]
        orow = [orow0, orow1]
        pa = [pa0, pa1]
        pb = [pb0, pb1]
        tot = [tot0, tot1]
        dma_out_s = [dma_out0, dma_out1]

        # dve_sem counts one per group (incremented by the per-group bias add)

        @block.gpsimd
        def _(gpsimd):
            gpsimd.dma_start(
                wt[:], bass.AP(tensor=w, offset=0, ap=[[0, P], [1, D]])
            ).then_inc(cset, 16)
            with nc.allow_non_contiguous_dma(reason="one-time 8KB bias load"):
                gpsimd.dma_start(
                    bias_t[:], bass.AP(tensor=bias, offset=0, ap=[[1, P], [P, NCHUNK]])
                ).then_inc(cset, 16)
            gpsimd.memset(ones_col[:], 1.0).then_inc(ones_sem, 1)
            for v in range(NS):
                s, p_ = v % BPC, v % 2
                xs = x[s].rearrange("(c p) d -> p c d", p=P)
                if v >= 2:
                    gpsimd.wait_ge(pe_pool, v - 1)  # xt[p_] free (pooling v-2 done)
                for g in range(NGROUP):
                    gpsimd.dma_start(
                        out=xt[p_][:, g * CPG : (g + 1) * CPG, :],
                        in_=xs[:, g * CPG : (g + 1) * CPG, :],
                    ).then_inc(dma_in_s[p_][g], 16)

        @block.sync
        def _(sync):
            for r in range(reps):
                rp = r % 2
                sync.wait_ge(act_out, BPC * (r + 1))  # all copies of rep r done
                sync.dma_start(out=out[:, :], in_=orow[rp][:]).then_inc(
                    dma_out_s[rp], 16
                )
            # drain: make sure every output store has landed before the
            # program retires (the runtime may otherwise read early)
            sync.wait_ge(dma_out_s[0], 16 * ((reps + 1) // 2))
            if reps > 1:
                sync.wait_ge(dma_out_s[1], 16 * (reps // 2))

        @block.vector
        def _(vector):
            vector.wait_ge(cset, 32)  # wt + bias_t loaded
            for v in range(NS):
                s, p_ = v % BPC, v % 2
                for g in range(NGROUP):
                    vector.wait_ge(dma_in_s[p_][g], 16 * (v // 2 + 1))
                    if g == 0 and v >= 2:
                        # score[p_] free (all exp groups of v-2 done)
                        vector.wait_ge(act_sem, NGROUP * (v - 1))
                    for c in range(g * CPG, (g + 1) * CPG):
                        sc = p_ * NCHUNK + c
                        nc.vector.scalar_tensor_tensor(
                            out=scrap[:, sc : sc + 1].broadcast_to((P, D)),
                            in0=xt[p_][:, c, :].bitcast(f32),
                            scalar=0.0,
                            in1=wt[:],
                            op0=Alu.bypass,
                            op1=Alu.mult,
                            accum_out=score[p_][:, c : c + 1],
                        ).then_inc(stt_sem, 1)
                    gs = slice(g * CPG, (g + 1) * CPG)
                    vector.wait_ge(stt_sem, NCHUNK * v + (g + 1) * CPG)
                    nc.vector.tensor_tensor(
                        out=score[p_][:, gs],
                        in0=score[p_][:, gs],
                        in1=bias_t[:, gs],
                        op=Alu.add,
                    ).then_inc(dve_sem, 1)
                if v >= 1:
                    pv, pp = v - 1, (v - 1) % 2
                    vector.wait_ge(pe_tot, pv + 1)
                    if pv >= 2:
                        vector.wait_ge(act_out, pv - 1)  # recip[pp] free (copy pv-2)
                    nc.vector.reciprocal(out=recip[pp][:], in_=tot[pp][:]).then_inc(
                        recip_sem, 1
                    )
            pv, pp = NS - 1, (NS - 1) % 2
            vector.wait_ge(pe_tot, pv + 1)
            if pv >= 2:
                vector.wait_ge(act_out, pv - 1)
            nc.vector.reciprocal(out=recip[pp][:], in_=tot[pp][:]).then_inc(
                recip_sem, 1
            )

        def _emit_copies(scalar, v):
            s, p_ = v % BPC, v % 2
            r, rp = v // BPC, (v // BPC) % 2
            scalar.wait_ge(pe_pool, v + 1)
            scalar.wait_ge(recip_sem, v + 1)
            if s == 0 and r >= 2:
                # orow[rp] free (batched out-DMA of rep r-2 done)
                scalar.wait_ge(dma_out_s[rp], 16 * (r // 2))
            o0 = s * D
            nc.scalar.activation(
                out=orow[rp][:, o0 : o0 + 512], in_=pa[p_][:], func=Copy,
                scale=recip[p_][:],
            )
            nc.scalar.activation(
                out=orow[rp][:, o0 + 512 : o0 + 1024], in_=pb[p_][:], func=Copy,
                scale=recip[p_][:],
            ).then_inc(act_out, 1)

        @block.scalar
        def _(scalar):
            for v in range(NS):
                s, p_ = v % BPC, v % 2
                for g in range(NGROUP):
                    gs = slice(g * CPG, (g + 1) * CPG)
                    scalar.wait_ge(dve_sem, NGROUP * v + g + 1)
                    nc.scalar.activation(
                        out=score[p_][:, gs], in_=score[p_][:, gs], func=Tanh
                    ).then_inc(act_i_sem, 1)
                    scalar.wait_ge(act_i_sem, NGROUP * v + g + 1)
                    if g == 0 and v >= 2:
                        scalar.wait_ge(pe_pool, v - 1)  # esc[p_] free (pooling v-2)
                    nc.scalar.activation(
                        out=esc[p_][:, gs],
                        in_=score[p_][:, gs],
                        func=Exp,
                        accum_out=sumexp[p_][:, g : g + 1],
                    ).then_inc(act_sem, 1)
                if v >= 1:
                    _emit_copies(scalar, v - 1)
            _emit_copies(scalar, NS - 1)

        @block.tensor
        def _(tensor):
            tensor.wait_ge(ones_sem, 1)  # ones ready
            for v in range(NS):
                s, p_ = v % BPC, v % 2
                for g in range(NGROUP):
                    tensor.wait_ge(act_sem, NGROUP * v + g + 1)
                    if g == 0 and v >= 2:
                        tensor.wait_ge(recip_sem, v - 1)  # tot[p_] free (recip v-2)
                        tensor.wait_ge(act_out, v - 1)  # pa/pb[p_] free (copies v-2)
                    mm_t = nc.tensor.matmul(
                        tot[p_][:],
                        sumexp[p_][:, g : g + 1],
                        ones_col[:],
                        start=(g == 0),
                        stop=(g == NGROUP - 1),
                    )
                    if g == NGROUP - 1:
                        mm_t.then_inc(pe_tot, 1)
                    for c in range(g * CPG, (g + 1) * CPG):
                        st, sp = c == 0, c == NCHUNK - 1
                        nc.tensor.matmul(
                            pa[p_][:], esc[p_][:, c : c + 1], xt[p_][:, c, 0:512],
                            start=st, stop=sp,
                        )
                        mm = nc.tensor.matmul(
                            pb[p_][:], esc[p_][:, c : c + 1], xt[p_][:, c, 512:1024],
                            start=st, stop=sp,
                        )
                mm.then_inc(pe_pool, 1)

    return nc


_NC_CACHE: dict = {}


def _build(reps: int = 1, ngroup: int = NGROUP) -> bass.Bass:
    key = (reps, ngroup)
    if key not in _NC_CACHE:
        _NC_CACHE[key] = _build_nc(reps, ngroup)
    return _NC_CACHE[key]


def _in_maps(x, w, b):
    return [
        {"x": x[c * BPC : (c + 1) * BPC], "w": w, "bias": b} for c in range(N_CORES)
    ]


def kernel(**inputs):
    x = np.ascontiguousarray(np.asarray(inputs["inputs"], dtype=np.float32))
    w = np.ascontiguousarray(np.asarray(inputs["att_weight"], dtype=np.float32))
    b = np.ascontiguousarray(np.asarray(inputs["att_bias"], dtype=np.float32))
    nc = _build()
    res = run_bass_kernel_spmd(nc, _in_maps(x, w, b), list(range(N_CORES)))
    return np.concatenate([r["out"] for r in res.results], axis=0)
